# revision 57
# baseline (speedup 1.0000x reference)
"""Trainium2 Bass kernel for nn_BayesianAtlas.

Strategy
--------
The module = tiny CNN encoder -> tiny deconv decoder -> 10 Euler steps of
20k template points advected through per-(t,batch) 16x16x2 velocity fields
via bilinear interpolation.  >97% of the work is the advection
(10 steps x 256 batches x 20000 points).

Encoder/decoder (~30 MFLOP total) run on host in numpy (exact f32 replica of
the jax reference).  The advection runs on 8 NeuronCores, data-parallel over
batch (32 batches/core).  Step t=0 is also done on the host (positions there
are the template for every batch, so it is one cheap vectorized bilinear);
the device runs steps 1..9.

Device formulation (no gathers, no clamps): hat(d) = relu(1-|d|) satisfies
the exact global identity hat(d) = (|d-1| - 2|d| + |d+1|)/2, so with C the
tridiagonal second-difference matrix (rows 1..14 only; hat_0/hat_15 never
fire since all coords stay in [1.49, 13.51]):
    interp(u,v)_c = sum_{k,l} |u-k| * (C vel_c C^T)[k,l] * |v-l|
The velocity tables are C-transformed on the host (same magnitude as vel,
perfectly conditioned), and the device consumes AV = |coord - grid| directly
as bilinear weights -- no clamped-hat (lerp) step exists at all.

Per core, points are packed into two half tiles per (group, chunk):
[72, w] bf16 with rows 0..63 = dX at partition (s%4)*16 + c*8 + g
(s = point-chunk 0..7, c = coordinate, g = batch-in-group 0..7) and rows
64..71 = the static base 3*x0 for (s%4, c) -- so one K=72 matmul emits
D = 3*dX + 3*x0 directly.  Per (t, group, column-chunk), per s-pair:
  mm_a (PE):  D[(g,j), p] = 3*dX + 3*x0       (K=72, bf16)
  abs (ACT):  AV = |D + (7.5-j)|              (per-partition bias, bf16 out)
  m3 (PE):    A_c = TBL_c^T @ AVV             (block-diag 64*DT*velC, bf16)
  prod (VEC): P = A_c * AVU                   (bf16 out, c-pair merged TT)
  m4 (PE):    R += SELQ^T @ P                 (sum over k, scatter to (s,c,g))
  id  (PE):   R += 64*dX (opens the R group; folds the state add into PSUM)
  upd (ACT):  dX = R * (1/64)                 (Copy activation, PSUM->SBUF)
(fp8 P with DoubleRow m4 was tried and is 10% faster, but the DVE's fp8
output flushes subnormals to zero, costing 25x in accuracy -- not worth it.)
m4 is emitted one s-pair behind its producers so the in-order PE queue never
head-of-line blocks on the DVE product.  Output = template + dX (host).
"""

import numpy as np

# ---------------------------------------------------------------- constants
B = 256
SG = 64
DG = 16
T = 11
LAT = 10
NPTS = 20000
DT = np.float32(1.0 / (T - 1))
NCORES = 8
BC = B // NCORES          # 32 batches per core
NM = 4                    # macro groups per core
G = 8                     # batches per macro group
NSTEPS = T - 1
NSTEPS_DEV = NSTEPS - 1   # t=0 on host
W = 2500                  # dX columns; point p of a batch: s = p // W, w = p % W
CHUNK = 500
NCHUNK = W // CHUNK
PSC = 64.0                # fp8 pre-scale for P (power of two, exact)

_COMPILED = None


def _to_bf16(x):
    import ml_dtypes
    return np.asarray(x, np.float32).astype(ml_dtypes.bfloat16)


def _to_f8(x):
    import ml_dtypes
    return np.asarray(x, np.float32).astype(ml_dtypes.float8_e4m3fn)


# ----------------------------------------------------- host encoder/decoder
def _conv2x2s2(x, w):
    N, C, H, Wd = x.shape
    xv = x.reshape(N, C, H // 2, 2, Wd // 2, 2)
    return np.einsum('ncidje,ocde->noij', xv, w, optimize=True).astype(np.float32)


def _convT2x2s2(x, w):
    # jax.lax.conv_transpose(..., 'VALID', ('NCHW','IOHW','NCHW')) flips the
    # kernel spatially relative to torch ConvTranspose2d semantics.
    N, C, H, Wd = x.shape
    wf = w[:, :, ::-1, ::-1]
    y = np.einsum('ncij,code->noidje', x, wf, optimize=True)
    return y.reshape(N, w.shape[1], 2 * H, 2 * Wd).astype(np.float32)


def _velocity_tables(inputs):
    x = inputs['observations'].astype(np.float32)
    for wk, bk in (('enc_w1', 'enc_b1'), ('enc_w2', 'enc_b2'),
                   ('enc_w3', 'enc_b3'), ('enc_w4', 'enc_b4')):
        x = np.tanh(_conv2x2s2(x, inputs[wk]) + inputs[bk][None, :, None, None]).astype(np.float32)
    x = x.reshape(x.shape[0], -1)
    z = (x @ inputs['enc_lin_w'].T + inputs['enc_lin_b']).astype(np.float32)

    scales = (np.arange(1, T, dtype=np.float32) * DT).astype(np.float32)
    z_all = (scales[:, None, None] * z[None]).reshape((T - 1) * B, LAT).astype(np.float32)

    h = np.tanh(z_all @ inputs['dec_lin_w'].T).astype(np.float32).reshape(-1, 16, 2, 2)
    h = np.tanh(_convT2x2s2(h, inputs['dec_w1'])).astype(np.float32)
    h = np.tanh(_convT2x2s2(h, inputs['dec_w2'])).astype(np.float32)
    v = _convT2x2s2(h, inputs['dec_w3'])
    # [T-1, B, i(u-dim), j(v-dim), c]
    return v.reshape(T - 1, B, 2, DG, DG).transpose(0, 1, 3, 4, 2)


# ------------------------------------------------------------- device build
def _build_kernel(nsteps=NSTEPS_DEV):
    from concourse import bacc, mybir, tile
    from concourse.bass import broadcast_tensor_aps

    f32 = mybir.dt.float32
    bf16 = mybir.dt.bfloat16
    Abs = mybir.ActivationFunctionType.Abs
    Copy = mybir.ActivationFunctionType.Copy
    Alu = mybir.AluOpType

    nc = bacc.Bacc("TRN2", target_bir_lowering=False, debug=False,
                   num_devices=NCORES)

    tbl_d = nc.dram_tensor('tbl', [128, nsteps * NM * 2 * 128], bf16, kind='ExternalInput')
    l1b_d = nc.dram_tensor('l1b', [72, 8 * 128], bf16, kind='ExternalInput')
    base_d = [nc.dram_tensor(f'base{hf}', [8, W], bf16, kind='ExternalInput')
              for hf in range(2)]
    dx0_d = [nc.dram_tensor(f'dx0_{m}', [128, W], bf16, kind='ExternalInput')
             for m in range(NM)]
    bias_d = nc.dram_tensor('bias', [128, 1], f32, kind='ExternalInput')
    selq_d = nc.dram_tensor('selq', [128, 16 * 128], bf16, kind='ExternalInput')
    seli_d = nc.dram_tensor('seli', [64, 2 * 128], bf16, kind='ExternalInput')
    xout_d = [nc.dram_tensor(f'xout{m}', [128, W], bf16, kind='ExternalOutput')
              for m in range(NM)]

    with tile.TileContext(nc) as tc:
        with (
            tc.tile_pool(name='const', bufs=1) as constp,
            tc.tile_pool(name='xs', bufs=1) as xsp,
            tc.tile_pool(name='da', bufs=3, space='PSUM') as dap,
            tc.tile_pool(name='rp', bufs=2, space='PSUM') as rpool,
            tc.tile_pool(name='avp', bufs=4) as avp,
            tc.tile_pool(name='pp', bufs=5) as pp,
        ):
            tbl = constp.tile([128, nsteps * NM * 2 * 128], bf16, tag='tbl')
            l1b = constp.tile([72, 8 * 128], bf16, tag='l1b')
            nc.sync.dma_start(l1b[:], l1b_d.ap())
            bias = constp.tile([128, 1], f32, tag='bias')
            nc.sync.dma_start(bias[:], bias_d.ap())
            selq = constp.tile([128, 16 * 128], bf16, tag='selq')
            nc.sync.dma_start(selq[:], selq_d.ap())
            seli = constp.tile([64, 2 * 128], bf16, tag='seli')
            nc.sync.dma_start(seli[:], seli_d.ap())

            # X[m][half][k]: rows 0..63 dX for s in (4*half..4*half+3),
            # rows 64..71 static base 3*x0 for (s%4, c)
            X = [[[xsp.tile([72, CHUNK], bf16, tag=f'x_{m}_{hf}_{k}',
                            name=f'x_{m}_{hf}_{k}')
                   for k in range(NCHUNK)] for hf in range(2)] for m in range(NM)]
            for m in range(NM):
                for hf in range(2):
                    for k in range(NCHUNK):
                        xt = X[m][hf][k]
                        nc.sync.dma_start(
                            xt[0:64, :],
                            dx0_d[m].ap()[64 * hf:64 * hf + 64,
                                          k * CHUNK:(k + 1) * CHUNK])
                        nc.sync.dma_start(
                            xt[64:72, :],
                            base_d[hf].ap()[:, k * CHUNK:(k + 1) * CHUNK])

            # stream the step tables behind the X inits so step-0 compute
            # starts as soon as its own slice lands
            TSTEP = NM * 2 * 128
            for t in range(nsteps):
                nc.sync.dma_start(tbl[:, t * TSTEP:(t + 1) * TSTEP],
                                  tbl_d.ap()[:, t * TSTEP:(t + 1) * TSTEP])

            for t in range(nsteps):
                for m in range(NM):
                    for k in range(NCHUNK):
                        cs = slice(0, CHUNK)
                        R = rpool.tile([128, CHUNK], f32, tag='r')
                        # open the R accumulation group with 64*dX
                        for hf in range(2):
                            nc.tensor.matmul(
                                R[:], seli[:, hf * 128:(hf + 1) * 128],
                                X[m][hf][k][0:64, cs],
                                start=(hf == 0), stop=False,
                                skip_group_check=True)

                        pend = []

                        def emit_m4(flush=False):
                            lag = 0 if flush else 2
                            while len(pend) > lag:
                                P, pr0, c = pend.pop(0)
                                for h in (0, 1):
                                    s = 2 * pr0 + h
                                    scol = (s * 2 + c) * 128
                                    rhs = P[:, h * 512:h * 512 + CHUNK]
                                    last = flush and not pend and h == 1
                                    nc.tensor.matmul(
                                        R[:], selq[:, scol:scol + 128], rhs,
                                        start=False, stop=last,
                                        skip_group_check=True)

                        for pr in range(4):
                            pr2 = pr // 2
                            win = X[m][pr2][k][0:72, cs]
                            WW = []
                            for uv in (1, 0):     # 0 = u (x, c=0 rows), 1 = v (y)
                                # 1024-wide so each half sits in its own psum bank
                                D = dap.tile([128, 1024], f32, tag='da')
                                for h in (0, 1):
                                    s = 2 * pr + h
                                    v = (s % 4) * 2 + uv
                                    nc.tensor.matmul(
                                        D[:, h * 512:h * 512 + CHUNK],
                                        l1b[0:72, v * 128:(v + 1) * 128],
                                        win, start=True, stop=True,
                                        skip_group_check=True)
                                AV = avp.tile([128, 2 * CHUNK], bf16, tag='av')
                                Dv = D[:].rearrange("p (h w) -> p h w", h=2)[:, :, 0:CHUNK]
                                nc.scalar.activation(AV[:], Dv, Abs, bias=bias[:], scale=1.0)
                                WW.append(AV)
                            WV, WU = WW
                            for c in (0, 1):
                                # both h into one A tile: same stationary tbl_c
                                A = dap.tile([128, 1024], f32, tag='da')
                                tcol = ((t * NM + m) * 2 + c) * 128
                                for h in (0, 1):
                                    nc.tensor.matmul(
                                        A[:, h * 512:h * 512 + CHUNK],
                                        tbl[:, tcol:tcol + 128],
                                        WV[:, h * CHUNK:(h + 1) * CHUNK],
                                        start=True, stop=True)
                                # merged product over both h: P = A * AVU
                                P = pp.tile([128, 1024], bf16, tag='p')
                                Av = A[:].rearrange("p (h2 w) -> p h2 w", h2=2)[:, :, 0:CHUNK]
                                Pv = P[:].rearrange("p (h2 w) -> p h2 w", h2=2)[:, :, 0:CHUNK]
                                wu = WU[:].rearrange("p (h2 w) -> p h2 w", h2=2)
                                nc.vector.tensor_tensor(Pv, Av, wu, Alu.mult)
                                pend.append((P, pr, c))
                                emit_m4()
                        emit_m4(flush=True)
                        for hf in range(2):
                            nc.scalar.activation(
                                X[m][hf][k][0:64, cs],
                                R[64 * hf:64 * hf + 64, cs],
                                Copy, bias=0.0, scale=1.0 / PSC)

            for m in range(NM):
                for hf in range(2):
                    for k in range(NCHUNK):
                        nc.sync.dma_start(
                            xout_d[m].ap()[64 * hf:64 * hf + 64,
                                           k * CHUNK:(k + 1) * CHUNK],
                            X[m][hf][k][0:64, :])

    nc.compile()
    return nc


def _get_compiled():
    global _COMPILED
    if _COMPILED is None:
        _COMPILED = _build_kernel()
    return _COMPILED


# ------------------------------------------------------------- host tensors
def _cmat():
    C = np.zeros((DG, DG), np.float32)
    for kk in range(DG):
        for jj in (kk - 1, kk, kk + 1):
            if 1 <= jj <= DG - 2:
                C[kk, jj] = 0.5 * (-2.0 if jj == kk else 1.0)
    return C


def _host_inputs(inputs):
    v_raw = _velocity_tables(inputs)   # [10, B, i, j, c]
    tp = inputs['template_points'].astype(np.float32)

    # abs-basis transform: hat_j(v) = (|v-(j-1)| - 2|v-j| + |v-(j+1)|)/2 for
    # j=1..14; hat_0/hat_15 vanish on the coord range, so their rows/cols drop.
    C = _cmat()
    v_all = np.einsum('ai,bj,tnijc->tnabc', C, C, v_raw).astype(np.float32)

    # ---- step t=0 on host: positions are the template for every batch ----
    u = 3.0 * tp[:, 0] + 7.5
    v = 3.0 * tp[:, 1] + 7.5
    i0 = np.clip(np.floor(u), 0, DG - 1).astype(np.int64)
    j0 = np.clip(np.floor(v), 0, DG - 1).astype(np.int64)
    i1 = np.clip(i0 + 1, 0, DG - 1)
    j1 = np.clip(j0 + 1, 0, DG - 1)
    fu = (u - i0)[None, :, None]
    fv = (v - j0)[None, :, None]
    vf = v_raw[0].reshape(B, DG * DG, 2)
    dx0 = DT * ((vf[:, i0 * DG + j0] * (1 - fu) * (1 - fv)
                 + vf[:, i0 * DG + j1] * (1 - fu) * fv
                 + vf[:, i1 * DG + j0] * fu * (1 - fv)
                 + vf[:, i1 * DG + j1] * fu * fv))   # [B, NPTS, 2]
    dx0 = _to_bf16(dx0).astype(np.float32)

    # base rows: base[hf][(s%4)*2 + c, j] = 3*x0[(4*hf + s%4)*W + j, c]
    u0 = 3.0 * tp                       # [NPTS, 2]
    bases = []
    for hf in range(2):
        bh = np.zeros((8, W), np.float32)
        for s4 in range(4):
            for c in range(2):
                p0 = (4 * hf + s4) * W
                bh[s4 * 2 + c] = u0[p0:p0 + W, c]
        bases.append(_to_bf16(bh))

    # mm_a stationary variants, K=72 (rows 0..63: s%4, c, g; rows 64..71 base):
    # L1B[(v//2)*16 + (v%2)*8 + g, v*128 + g*16 + j] = 3, and
    # L1B[64 + v, v*128 + :] = 1  (injects base row (s%4, c=uv))
    # where variant v = (s%4)*2 + uv  (uv: 0 = u rows (c=0), 1 = v rows (c=1))
    l1b = np.zeros((72, 8 * 128), np.float32)
    for vv in range(8):
        roff = (vv // 2) * 16 + (vv % 2) * 8
        for g in range(8):
            l1b[roff + g, vv * 128 + g * 16:vv * 128 + g * 16 + 16] = 3.0
        l1b[64 + vv, vv * 128:(vv + 1) * 128] = 1.0

    biasv = np.zeros((128, 1), np.float32)
    biasv[:, 0] = 7.5 - (np.arange(128) % 16)

    # m4 stationaries: SELQ[(g*16+i), (s*2+c)*128 + (s*16+c*8+g)] = 1
    selq = np.zeros((128, 16 * 128), np.float32)
    for s in range(8):
        for c in range(2):
            base = (s * 2 + c) * 128
            for g in range(8):
                selq[g * 16:(g + 1) * 16, base + s * 16 + c * 8 + g] = 1.0

    # identity-add stationaries: R[64*hf + r] += PSC * xt[r]
    seli = np.zeros((64, 2 * 128), np.float32)
    for hf in range(2):
        for r in range(64):
            seli[r, hf * 128 + 64 * hf + r] = PSC

    # per-core block-diag tables for device steps t=1..9 (scaled by PSC)
    # TBL[(g*16+j), ((t*NM+m)*2+c)*128 + g*16+i] = PSC * DT * velC[b][i, j, c]
    vv_ = v_all[1:].reshape(NSTEPS_DEV, NCORES, NM, G, DG, DG, 2)
    tbls = []
    for core in range(NCORES):
        tblc = np.zeros((NSTEPS_DEV, NM, 2, G, 16, G, 16), np.float32)
        for g in range(G):
            tblc[:, :, :, g, :, g, :] = (vv_[:, core, :, g].transpose(0, 1, 4, 3, 2)
                                         * (DT * PSC))
        tbl = tblc.transpose(3, 4, 0, 1, 2, 5, 6).reshape(128, NSTEPS_DEV * NM * 2 * 128)
        tbls.append(_to_bf16(tbl))

    # per-core dx0 in X-tile layout: dx0_m[s*16+c*8+g, s-col] for batch core*32+m*8+g
    dx0s = []
    for core in range(NCORES):
        percore = []
        for m in range(NM):
            dm = np.zeros((128, W), np.float32)
            for g in range(G):
                b = core * BC + m * G + g
                r = dx0[b].reshape(8, W, 2)          # [s, w, c]
                for s in range(8):
                    for c in range(2):
                        dm[s * 16 + c * 8 + g] = r[s, :, c]
            percore.append(_to_bf16(dm))
        dx0s.append(percore)

    return tbls, bases, _to_bf16(l1b), biasv, _to_bf16(selq), _to_bf16(seli), dx0s, tp


LAST_RES = None


def kernel(**inputs):
    global LAST_RES
    import os
    inputs = {k: np.asarray(v) for k, v in inputs.items()}
    from concourse.bass_utils import run_bass_kernel_spmd

    nc = _get_compiled()
    tbls, bases, l1b, biasv, selq, seli, dx0s, tp = _host_inputs(inputs)

    in_maps = []
    for core in range(NCORES):
        im = {'tbl': tbls[core], 'base0': bases[0], 'base1': bases[1],
              'l1b': l1b, 'bias': biasv, 'selq': selq, 'seli': seli}
        for m in range(NM):
            im[f'dx0_{m}'] = dx0s[core][m]
        in_maps.append(im)
    tmpdir = os.environ.get('BASS_TRACE_DIR') or None
    if tmpdir:
        os.makedirs(tmpdir, exist_ok=True)
    res = run_bass_kernel_spmd(nc, in_maps, list(range(NCORES)), tmpdir=tmpdir)
    LAST_RES = res

    out = np.empty((B, NPTS, 2), np.float32)
    for core in range(NCORES):
        for m in range(NM):
            xm = np.asarray(res.results[core][f'xout{m}']).astype(np.float32)
            rm = xm.reshape(8, 2, 8, W)                         # [s, c, g, w]
            b0 = core * BC + m * G
            out[b0:b0 + G] = tp[None] + rm.transpose(2, 0, 3, 1).reshape(G, NPTS, 2)
    return out


# revision 59
# speedup vs baseline: 1.1990x; 1.1990x over previous
"""Trainium2 Bass kernel for nn_BayesianAtlas.

Strategy
--------
The module = tiny CNN encoder -> tiny deconv decoder -> 10 Euler steps of
20k template points advected through per-(t,batch) 16x16x2 velocity fields
via bilinear interpolation.  >97% of the work is the advection
(10 steps x 256 batches x 20000 points).

Encoder/decoder (~30 MFLOP total) run on host in numpy (exact f32 replica of
the jax reference).  The advection runs on 8 NeuronCores, data-parallel over
batch (32 batches/core).  Step t=0 is also done on the host (positions there
are the template for every batch, so it is one cheap vectorized bilinear);
the device runs steps 1..9.

Device formulation (no gathers, no clamps): hat(d) = relu(1-|d|) satisfies
the exact global identity hat(d) = (|d-1| - 2|d| + |d+1|)/2, so with C the
tridiagonal second-difference matrix (rows 1..14 only; hat_0/hat_15 never
fire since all coords stay in [1.49, 13.51]):
    interp(u,v)_c = sum_{k,l} |u-k| * (C vel_c C^T)[k,l] * |v-l|
The velocity tables are C-transformed on the host (same magnitude as vel,
perfectly conditioned), and the device consumes AV = |coord - grid| directly
as bilinear weights -- no clamped-hat (lerp) step exists at all.

Per core, points are packed into two half tiles per (group, chunk):
[72, w] bf16 with rows 0..63 = dX at partition (s%4)*16 + c*8 + g
(s = point-chunk 0..7, c = coordinate, g = batch-in-group 0..7) and rows
64..71 = the static base 3*x0 for (s%4, c) -- so one K=72 matmul emits
D = 3*dX + 3*x0 directly.  Per (t, group, column-chunk), per s-pair:
  mm_a (PE):  D[(g,j), p] = 3*dX + 3*x0       (K=72, bf16)
  abs (ACT):  AV = |D + (7.5-j)|              (per-partition bias, bf16 out)
  m3 (PE):    A_c = TBL_c^T @ AVV             (block-diag 64*DT*velC, bf16)
  prod (VEC): P = A_c * AVU                   (bf16 out, c-pair merged TT)
  m4 (PE):    R += SELQ^T @ P                 (sum over k, scatter to (s,c,g))
  id  (PE):   R += 64*dX (opens the R group; folds the state add into PSUM)
  upd (ACT):  dX = R * (1/64)                 (Copy activation, PSUM->SBUF)
(fp8 P with DoubleRow m4 was tried and is 10% faster, but the DVE's fp8
output flushes subnormals to zero, costing 25x in accuracy -- not worth it.)
m4 is emitted one s-pair behind its producers so the in-order PE queue never
head-of-line blocks on the DVE product.  Output = template + dX (host).
"""

import numpy as np

# ---------------------------------------------------------------- constants
B = 256
SG = 64
DG = 16
T = 11
LAT = 10
NPTS = 20000
DT = np.float32(1.0 / (T - 1))
NCORES = 8
BC = B // NCORES          # 32 batches per core
NM = 4                    # macro groups per core
G = 8                     # batches per macro group
NSTEPS = T - 1
NSTEPS_DEV = NSTEPS - 1   # t=0 on host
W = 2500                  # dX columns; point p of a batch: s = p // W, w = p % W
CHUNK = 500
NCHUNK = W // CHUNK
PSC = 64.0                # fp8 pre-scale for P (power of two, exact)

_COMPILED = None


def _to_bf16(x):
    import ml_dtypes
    return np.asarray(x, np.float32).astype(ml_dtypes.bfloat16)


def _to_f8(x):
    import ml_dtypes
    return np.asarray(x, np.float32).astype(ml_dtypes.float8_e4m3fn)


# ----------------------------------------------------- host encoder/decoder
def _conv2x2s2(x, w):
    N, C, H, Wd = x.shape
    xv = x.reshape(N, C, H // 2, 2, Wd // 2, 2)
    return np.einsum('ncidje,ocde->noij', xv, w, optimize=True).astype(np.float32)


def _convT2x2s2(x, w):
    # jax.lax.conv_transpose(..., 'VALID', ('NCHW','IOHW','NCHW')) flips the
    # kernel spatially relative to torch ConvTranspose2d semantics.
    N, C, H, Wd = x.shape
    wf = w[:, :, ::-1, ::-1]
    y = np.einsum('ncij,code->noidje', x, wf, optimize=True)
    return y.reshape(N, w.shape[1], 2 * H, 2 * Wd).astype(np.float32)


def _velocity_tables(inputs):
    x = inputs['observations'].astype(np.float32)
    for wk, bk in (('enc_w1', 'enc_b1'), ('enc_w2', 'enc_b2'),
                   ('enc_w3', 'enc_b3'), ('enc_w4', 'enc_b4')):
        x = np.tanh(_conv2x2s2(x, inputs[wk]) + inputs[bk][None, :, None, None]).astype(np.float32)
    x = x.reshape(x.shape[0], -1)
    z = (x @ inputs['enc_lin_w'].T + inputs['enc_lin_b']).astype(np.float32)

    scales = (np.arange(1, T, dtype=np.float32) * DT).astype(np.float32)
    z_all = (scales[:, None, None] * z[None]).reshape((T - 1) * B, LAT).astype(np.float32)

    h = np.tanh(z_all @ inputs['dec_lin_w'].T).astype(np.float32).reshape(-1, 16, 2, 2)
    h = np.tanh(_convT2x2s2(h, inputs['dec_w1'])).astype(np.float32)
    h = np.tanh(_convT2x2s2(h, inputs['dec_w2'])).astype(np.float32)
    v = _convT2x2s2(h, inputs['dec_w3'])
    # [T-1, B, i(u-dim), j(v-dim), c]
    return v.reshape(T - 1, B, 2, DG, DG).transpose(0, 1, 3, 4, 2)


# ------------------------------------------------------------- device build
def _build_kernel(nsteps=NSTEPS_DEV):
    from concourse import bacc, mybir, tile
    from concourse.bass import broadcast_tensor_aps

    f32 = mybir.dt.float32
    bf16 = mybir.dt.bfloat16
    Abs = mybir.ActivationFunctionType.Abs
    Copy = mybir.ActivationFunctionType.Copy
    Alu = mybir.AluOpType

    nc = bacc.Bacc("TRN2", target_bir_lowering=False, debug=False,
                   num_devices=NCORES)

    tbl_d = nc.dram_tensor('tbl', [128, nsteps * NM * 2 * 128], bf16, kind='ExternalInput')
    l1b_d = nc.dram_tensor('l1b', [72, 8 * 128], bf16, kind='ExternalInput')
    base_d = [nc.dram_tensor(f'base{hf}', [8, W], bf16, kind='ExternalInput')
              for hf in range(2)]
    dx0_d = [nc.dram_tensor(f'dx0_{m}', [128, W], bf16, kind='ExternalInput')
             for m in range(NM)]
    bias_d = nc.dram_tensor('bias', [128, 1], f32, kind='ExternalInput')
    selq_d = nc.dram_tensor('selq', [128, 16 * 128], bf16, kind='ExternalInput')
    seli_d = nc.dram_tensor('seli', [64, 2 * 128], bf16, kind='ExternalInput')
    xout_d = [nc.dram_tensor(f'xout{m}', [128, W], bf16, kind='ExternalOutput')
              for m in range(NM)]

    with tile.TileContext(nc) as tc:
        with (
            tc.tile_pool(name='const', bufs=1) as constp,
            tc.tile_pool(name='xs', bufs=1) as xsp,
            tc.tile_pool(name='da', bufs=3, space='PSUM') as dap,
            tc.tile_pool(name='rp', bufs=2, space='PSUM') as rpool,
            tc.tile_pool(name='avp', bufs=4) as avp,
            tc.tile_pool(name='pp', bufs=4) as pp,
        ):
            TSTEP = NM * 2 * 128
            tbls_t = [constp.tile([128, TSTEP], bf16, tag=f'tbl{t}', name=f'tbl{t}')
                      for t in range(nsteps)]
            l1b = constp.tile([72, 8 * 128], bf16, tag='l1b')
            nc.sync.dma_start(l1b[:], l1b_d.ap())
            bias = constp.tile([128, 1], f32, tag='bias')
            nc.sync.dma_start(bias[:], bias_d.ap())
            selq = constp.tile([128, 16 * 128], bf16, tag='selq')
            nc.sync.dma_start(selq[:], selq_d.ap())
            seli = constp.tile([64, 2 * 128], bf16, tag='seli')
            nc.sync.dma_start(seli[:], seli_d.ap())

            # X[m][half][k]: rows 0..63 dX for s in (4*half..4*half+3),
            # rows 64..71 static base 3*x0 for (s%4, c)
            X = [[[xsp.tile([72, CHUNK], bf16, tag=f'x_{m}_{hf}_{k}',
                            name=f'x_{m}_{hf}_{k}')
                   for k in range(NCHUNK)] for hf in range(2)] for m in range(NM)]
            for m in range(NM):
                for hf in range(2):
                    for k in range(NCHUNK):
                        xt = X[m][hf][k]
                        nc.sync.dma_start(
                            xt[0:64, :],
                            dx0_d[m].ap()[64 * hf:64 * hf + 64,
                                          k * CHUNK:(k + 1) * CHUNK])
                        nc.sync.dma_start(
                            xt[64:72, :],
                            base_d[hf].ap()[:, k * CHUNK:(k + 1) * CHUNK])

            for t in range(nsteps):
                nc.sync.dma_start(tbls_t[t][:],
                                  tbl_d.ap()[:, t * TSTEP:(t + 1) * TSTEP])

            for t in range(nsteps):
                for m in range(NM):
                    for k in range(NCHUNK):
                        cs = slice(0, CHUNK)
                        R = rpool.tile([128, CHUNK], f32, tag='r')
                        # open the R accumulation group with 64*dX
                        for hf in range(2):
                            nc.tensor.matmul(
                                R[:], seli[:, hf * 128:(hf + 1) * 128],
                                X[m][hf][k][0:64, cs],
                                start=(hf == 0), stop=False,
                                skip_group_check=True)

                        pend = []

                        def emit_m4(flush=False):
                            lag = 0 if flush else 2
                            while len(pend) > lag:
                                P, pr0, c = pend.pop(0)
                                for h in (0, 1):
                                    s = 2 * pr0 + h
                                    scol = (s * 2 + c) * 128
                                    rhs = P[:, h * 512:h * 512 + CHUNK]
                                    last = flush and not pend and h == 1
                                    nc.tensor.matmul(
                                        R[:], selq[:, scol:scol + 128], rhs,
                                        start=False, stop=last,
                                        skip_group_check=True)

                        for pr in range(4):
                            pr2 = pr // 2
                            win = X[m][pr2][k][0:72, cs]
                            WW = []
                            for uv in (1, 0):     # 0 = u (x, c=0 rows), 1 = v (y)
                                # 1024-wide so each half sits in its own psum bank
                                D = dap.tile([128, 1024], f32, tag='da')
                                for h in (0, 1):
                                    s = 2 * pr + h
                                    v = (s % 4) * 2 + uv
                                    nc.tensor.matmul(
                                        D[:, h * 512:h * 512 + CHUNK],
                                        l1b[0:72, v * 128:(v + 1) * 128],
                                        win, start=True, stop=True,
                                        skip_group_check=True)
                                AV = avp.tile([128, 2 * CHUNK], bf16, tag='av')
                                Dv = D[:].rearrange("p (h w) -> p h w", h=2)[:, :, 0:CHUNK]
                                nc.scalar.activation(AV[:], Dv, Abs, bias=bias[:], scale=1.0)
                                WW.append(AV)
                            WV, WU = WW
                            for c in (0, 1):
                                # both h into one A tile: same stationary tbl_c
                                A = dap.tile([128, 1024], f32, tag='da')
                                tcol = (m * 2 + c) * 128
                                for h in (0, 1):
                                    nc.tensor.matmul(
                                        A[:, h * 512:h * 512 + CHUNK],
                                        tbls_t[t][:, tcol:tcol + 128],
                                        WV[:, h * CHUNK:(h + 1) * CHUNK],
                                        start=True, stop=True)
                                # merged product over both h: P = A * AVU
                                P = pp.tile([128, 1024], bf16, tag='p')
                                Av = A[:].rearrange("p (h2 w) -> p h2 w", h2=2)[:, :, 0:CHUNK]
                                Pv = P[:].rearrange("p (h2 w) -> p h2 w", h2=2)[:, :, 0:CHUNK]
                                wu = WU[:].rearrange("p (h2 w) -> p h2 w", h2=2)
                                nc.vector.tensor_tensor(Pv, Av, wu, Alu.mult)
                                pend.append((P, pr, c))
                                emit_m4()
                        emit_m4(flush=True)
                        for hf in range(2):
                            nc.scalar.activation(
                                X[m][hf][k][0:64, cs],
                                R[64 * hf:64 * hf + 64, cs],
                                Copy, bias=0.0, scale=1.0 / PSC)

            for m in range(NM):
                for hf in range(2):
                    for k in range(NCHUNK):
                        nc.sync.dma_start(
                            xout_d[m].ap()[64 * hf:64 * hf + 64,
                                           k * CHUNK:(k + 1) * CHUNK],
                            X[m][hf][k][0:64, :])

    nc.compile()
    return nc


def _get_compiled():
    global _COMPILED
    if _COMPILED is None:
        _COMPILED = _build_kernel()
    return _COMPILED


# ------------------------------------------------------------- host tensors
def _cmat():
    C = np.zeros((DG, DG), np.float32)
    for kk in range(DG):
        for jj in (kk - 1, kk, kk + 1):
            if 1 <= jj <= DG - 2:
                C[kk, jj] = 0.5 * (-2.0 if jj == kk else 1.0)
    return C


def _host_inputs(inputs):
    v_raw = _velocity_tables(inputs)   # [10, B, i, j, c]
    tp = inputs['template_points'].astype(np.float32)

    # abs-basis transform: hat_j(v) = (|v-(j-1)| - 2|v-j| + |v-(j+1)|)/2 for
    # j=1..14; hat_0/hat_15 vanish on the coord range, so their rows/cols drop.
    C = _cmat()
    v_all = np.einsum('ai,bj,tnijc->tnabc', C, C, v_raw).astype(np.float32)

    # ---- step t=0 on host: positions are the template for every batch ----
    u = 3.0 * tp[:, 0] + 7.5
    v = 3.0 * tp[:, 1] + 7.5
    i0 = np.clip(np.floor(u), 0, DG - 1).astype(np.int64)
    j0 = np.clip(np.floor(v), 0, DG - 1).astype(np.int64)
    i1 = np.clip(i0 + 1, 0, DG - 1)
    j1 = np.clip(j0 + 1, 0, DG - 1)
    fu = (u - i0)[None, :, None]
    fv = (v - j0)[None, :, None]
    vf = v_raw[0].reshape(B, DG * DG, 2)
    dx0 = DT * ((vf[:, i0 * DG + j0] * (1 - fu) * (1 - fv)
                 + vf[:, i0 * DG + j1] * (1 - fu) * fv
                 + vf[:, i1 * DG + j0] * fu * (1 - fv)
                 + vf[:, i1 * DG + j1] * fu * fv))   # [B, NPTS, 2]
    dx0 = _to_bf16(dx0).astype(np.float32)

    # base rows: base[hf][(s%4)*2 + c, j] = 3*x0[(4*hf + s%4)*W + j, c]
    u0 = 3.0 * tp                       # [NPTS, 2]
    bases = []
    for hf in range(2):
        bh = np.zeros((8, W), np.float32)
        for s4 in range(4):
            for c in range(2):
                p0 = (4 * hf + s4) * W
                bh[s4 * 2 + c] = u0[p0:p0 + W, c]
        bases.append(_to_bf16(bh))

    # mm_a stationary variants, K=72 (rows 0..63: s%4, c, g; rows 64..71 base):
    # L1B[(v//2)*16 + (v%2)*8 + g, v*128 + g*16 + j] = 3, and
    # L1B[64 + v, v*128 + :] = 1  (injects base row (s%4, c=uv))
    # where variant v = (s%4)*2 + uv  (uv: 0 = u rows (c=0), 1 = v rows (c=1))
    l1b = np.zeros((72, 8 * 128), np.float32)
    for vv in range(8):
        roff = (vv // 2) * 16 + (vv % 2) * 8
        for g in range(8):
            l1b[roff + g, vv * 128 + g * 16:vv * 128 + g * 16 + 16] = 3.0
        l1b[64 + vv, vv * 128:(vv + 1) * 128] = 1.0

    biasv = np.zeros((128, 1), np.float32)
    biasv[:, 0] = 7.5 - (np.arange(128) % 16)

    # m4 stationaries: SELQ[(g*16+i), (s*2+c)*128 + (s*16+c*8+g)] = 1
    selq = np.zeros((128, 16 * 128), np.float32)
    for s in range(8):
        for c in range(2):
            base = (s * 2 + c) * 128
            for g in range(8):
                selq[g * 16:(g + 1) * 16, base + s * 16 + c * 8 + g] = 1.0

    # identity-add stationaries: R[64*hf + r] += PSC * xt[r]
    seli = np.zeros((64, 2 * 128), np.float32)
    for hf in range(2):
        for r in range(64):
            seli[r, hf * 128 + 64 * hf + r] = PSC

    # per-core block-diag tables for device steps t=1..9 (scaled by PSC)
    # TBL[(g*16+j), ((t*NM+m)*2+c)*128 + g*16+i] = PSC * DT * velC[b][i, j, c]
    vv_ = v_all[1:].reshape(NSTEPS_DEV, NCORES, NM, G, DG, DG, 2)
    tbls = []
    for core in range(NCORES):
        tblc = np.zeros((NSTEPS_DEV, NM, 2, G, 16, G, 16), np.float32)
        for g in range(G):
            tblc[:, :, :, g, :, g, :] = (vv_[:, core, :, g].transpose(0, 1, 4, 3, 2)
                                         * (DT * PSC))
        tbl = tblc.transpose(3, 4, 0, 1, 2, 5, 6).reshape(128, NSTEPS_DEV * NM * 2 * 128)
        tbls.append(_to_bf16(tbl))

    # per-core dx0 in X-tile layout: dx0_m[s*16+c*8+g, s-col] for batch core*32+m*8+g
    dx0s = []
    for core in range(NCORES):
        percore = []
        for m in range(NM):
            dm = np.zeros((128, W), np.float32)
            for g in range(G):
                b = core * BC + m * G + g
                r = dx0[b].reshape(8, W, 2)          # [s, w, c]
                for s in range(8):
                    for c in range(2):
                        dm[s * 16 + c * 8 + g] = r[s, :, c]
            percore.append(_to_bf16(dm))
        dx0s.append(percore)

    return tbls, bases, _to_bf16(l1b), biasv, _to_bf16(selq), _to_bf16(seli), dx0s, tp


LAST_RES = None


def kernel(**inputs):
    global LAST_RES
    import os
    inputs = {k: np.asarray(v) for k, v in inputs.items()}
    from concourse.bass_utils import run_bass_kernel_spmd

    nc = _get_compiled()
    tbls, bases, l1b, biasv, selq, seli, dx0s, tp = _host_inputs(inputs)

    in_maps = []
    for core in range(NCORES):
        im = {'tbl': tbls[core], 'base0': bases[0], 'base1': bases[1],
              'l1b': l1b, 'bias': biasv, 'selq': selq, 'seli': seli}
        for m in range(NM):
            im[f'dx0_{m}'] = dx0s[core][m]
        in_maps.append(im)
    tmpdir = os.environ.get('BASS_TRACE_DIR') or None
    if tmpdir:
        os.makedirs(tmpdir, exist_ok=True)
    res = run_bass_kernel_spmd(nc, in_maps, list(range(NCORES)), tmpdir=tmpdir)
    LAST_RES = res

    out = np.empty((B, NPTS, 2), np.float32)
    for core in range(NCORES):
        for m in range(NM):
            xm = np.asarray(res.results[core][f'xout{m}']).astype(np.float32)
            rm = xm.reshape(8, 2, 8, W)                         # [s, c, g, w]
            b0 = core * BC + m * G
            out[b0:b0 + G] = tp[None] + rm.transpose(2, 0, 3, 1).reshape(G, NPTS, 2)
    return out


# revision 60
# speedup vs baseline: 1.2172x; 1.0152x over previous
"""Trainium2 Bass kernel for nn_BayesianAtlas.

Strategy
--------
The module = tiny CNN encoder -> tiny deconv decoder -> 10 Euler steps of
20k template points advected through per-(t,batch) 16x16x2 velocity fields
via bilinear interpolation.  >97% of the work is the advection
(10 steps x 256 batches x 20000 points).

Encoder/decoder (~30 MFLOP total) run on host in numpy (exact f32 replica of
the jax reference).  The advection runs on 8 NeuronCores, data-parallel over
batch (32 batches/core).  Step t=0 is also done on the host (positions there
are the template for every batch, so it is one cheap vectorized bilinear);
the device runs steps 1..9.

Device formulation (no gathers, no clamps): hat(d) = relu(1-|d|) satisfies
the exact global identity hat(d) = (|d-1| - 2|d| + |d+1|)/2, so with C the
tridiagonal second-difference matrix (rows 1..14 only; hat_0/hat_15 never
fire since all coords stay in [1.49, 13.51]):
    interp(u,v)_c = sum_{k,l} |u-k| * (C vel_c C^T)[k,l] * |v-l|
The velocity tables are C-transformed on the host (same magnitude as vel,
perfectly conditioned), and the device consumes AV = |coord - grid| directly
as bilinear weights -- no clamped-hat (lerp) step exists at all.

Per core, points are packed into two half tiles per (group, chunk):
[72, w] bf16 with rows 0..63 = dX at partition (s%4)*16 + c*8 + g
(s = point-chunk 0..7, c = coordinate, g = batch-in-group 0..7) and rows
64..71 = the static base 3*x0 for (s%4, c) -- so one K=72 matmul emits
D = 3*dX + 3*x0 directly.  Per (t, group, column-chunk), per s-pair:
  mm_a (PE):  D[(g,j), p] = 3*dX + 3*x0       (K=72, bf16)
  abs (ACT):  AV = |D + (7.5-j)|              (per-partition bias, bf16 out)
  m3 (PE):    A_c = TBL_c^T @ AVV             (block-diag 64*DT*velC, bf16)
  prod (VEC): P = A_c * AVU                   (bf16 out, c-pair merged TT)
  m4 (PE):    R += SELQ^T @ P                 (sum over k, scatter to (s,c,g))
  id  (PE):   R += 64*dX (opens the R group; folds the state add into PSUM)
  upd (ACT):  dX = R * (1/64)                 (Copy activation, PSUM->SBUF)
(fp8 P with DoubleRow m4 was tried and is 10% faster, but the DVE's fp8
output flushes subnormals to zero, costing 25x in accuracy -- not worth it.)
m4 is emitted one s-pair behind its producers so the in-order PE queue never
head-of-line blocks on the DVE product.  Output = template + dX (host).
"""

import numpy as np

# ---------------------------------------------------------------- constants
B = 256
SG = 64
DG = 16
T = 11
LAT = 10
NPTS = 20000
DT = np.float32(1.0 / (T - 1))
NCORES = 8
BC = B // NCORES          # 32 batches per core
NM = 4                    # macro groups per core
G = 8                     # batches per macro group
NSTEPS = T - 1
NSTEPS_DEV = NSTEPS - 1   # t=0 on host
W = 2500                  # dX columns; point p of a batch: s = p // W, w = p % W
CHUNK = 500
NCHUNK = W // CHUNK
PSC = 64.0                # fp8 pre-scale for P (power of two, exact)

_COMPILED = None


def _to_bf16(x):
    import ml_dtypes
    return np.asarray(x, np.float32).astype(ml_dtypes.bfloat16)


def _to_f8(x):
    import ml_dtypes
    return np.asarray(x, np.float32).astype(ml_dtypes.float8_e4m3fn)


# ----------------------------------------------------- host encoder/decoder
def _conv2x2s2(x, w):
    N, C, H, Wd = x.shape
    xv = x.reshape(N, C, H // 2, 2, Wd // 2, 2)
    return np.einsum('ncidje,ocde->noij', xv, w, optimize=True).astype(np.float32)


def _convT2x2s2(x, w):
    # jax.lax.conv_transpose(..., 'VALID', ('NCHW','IOHW','NCHW')) flips the
    # kernel spatially relative to torch ConvTranspose2d semantics.
    N, C, H, Wd = x.shape
    wf = w[:, :, ::-1, ::-1]
    y = np.einsum('ncij,code->noidje', x, wf, optimize=True)
    return y.reshape(N, w.shape[1], 2 * H, 2 * Wd).astype(np.float32)


def _velocity_tables(inputs):
    x = inputs['observations'].astype(np.float32)
    for wk, bk in (('enc_w1', 'enc_b1'), ('enc_w2', 'enc_b2'),
                   ('enc_w3', 'enc_b3'), ('enc_w4', 'enc_b4')):
        x = np.tanh(_conv2x2s2(x, inputs[wk]) + inputs[bk][None, :, None, None]).astype(np.float32)
    x = x.reshape(x.shape[0], -1)
    z = (x @ inputs['enc_lin_w'].T + inputs['enc_lin_b']).astype(np.float32)

    scales = (np.arange(1, T, dtype=np.float32) * DT).astype(np.float32)
    z_all = (scales[:, None, None] * z[None]).reshape((T - 1) * B, LAT).astype(np.float32)

    h = np.tanh(z_all @ inputs['dec_lin_w'].T).astype(np.float32).reshape(-1, 16, 2, 2)
    h = np.tanh(_convT2x2s2(h, inputs['dec_w1'])).astype(np.float32)
    h = np.tanh(_convT2x2s2(h, inputs['dec_w2'])).astype(np.float32)
    v = _convT2x2s2(h, inputs['dec_w3'])
    # [T-1, B, i(u-dim), j(v-dim), c]
    return v.reshape(T - 1, B, 2, DG, DG).transpose(0, 1, 3, 4, 2)


# ------------------------------------------------------------- device build
def _build_kernel(nsteps=NSTEPS_DEV):
    from concourse import bacc, mybir, tile
    from concourse.bass import broadcast_tensor_aps

    f32 = mybir.dt.float32
    bf16 = mybir.dt.bfloat16
    Abs = mybir.ActivationFunctionType.Abs
    Copy = mybir.ActivationFunctionType.Copy
    Alu = mybir.AluOpType

    nc = bacc.Bacc("TRN2", target_bir_lowering=False, debug=False,
                   num_devices=NCORES)

    tbl_d = nc.dram_tensor('tbl', [128, nsteps * NM * 2 * 128], bf16, kind='ExternalInput')
    l1b_d = nc.dram_tensor('l1b', [72, 8 * 128], bf16, kind='ExternalInput')
    base_d = [nc.dram_tensor(f'base{hf}', [8, W], bf16, kind='ExternalInput')
              for hf in range(2)]
    dx0_d = [nc.dram_tensor(f'dx0_{m}', [128, W], bf16, kind='ExternalInput')
             for m in range(NM)]
    bias_d = nc.dram_tensor('bias', [128, 1], f32, kind='ExternalInput')
    selq_d = nc.dram_tensor('selq', [128, 16 * 128], bf16, kind='ExternalInput')
    seli_d = nc.dram_tensor('seli', [64, 2 * 128], bf16, kind='ExternalInput')
    xout_d = [nc.dram_tensor(f'xout{m}', [128, W], bf16, kind='ExternalOutput')
              for m in range(NM)]

    with tile.TileContext(nc) as tc:
        with (
            tc.tile_pool(name='const', bufs=1) as constp,
            tc.tile_pool(name='xs', bufs=1) as xsp,
            tc.tile_pool(name='da', bufs=3, space='PSUM') as dap,
            tc.tile_pool(name='rp', bufs=2, space='PSUM') as rpool,
            tc.tile_pool(name='avp', bufs=4) as avp,
            tc.tile_pool(name='pp', bufs=5) as pp,
        ):
            tbl = constp.tile([128, nsteps * NM * 2 * 128], bf16, tag='tbl')
            nc.sync.dma_start(tbl[:], tbl_d.ap())
            l1b = constp.tile([72, 8 * 128], bf16, tag='l1b')
            nc.sync.dma_start(l1b[:], l1b_d.ap())
            bias = constp.tile([128, 1], f32, tag='bias')
            nc.sync.dma_start(bias[:], bias_d.ap())
            selq = constp.tile([128, 16 * 128], bf16, tag='selq')
            nc.sync.dma_start(selq[:], selq_d.ap())
            seli = constp.tile([64, 2 * 128], bf16, tag='seli')
            nc.sync.dma_start(seli[:], seli_d.ap())

            # X[m][half][k]: rows 0..63 dX for s in (4*half..4*half+3),
            # rows 64..71 static base 3*x0 for (s%4, c)
            X = [[[xsp.tile([72, CHUNK], bf16, tag=f'x_{m}_{hf}_{k}',
                            name=f'x_{m}_{hf}_{k}')
                   for k in range(NCHUNK)] for hf in range(2)] for m in range(NM)]
            for m in range(NM):
                for hf in range(2):
                    for k in range(NCHUNK):
                        xt = X[m][hf][k]
                        nc.sync.dma_start(
                            xt[0:64, :],
                            dx0_d[m].ap()[64 * hf:64 * hf + 64,
                                          k * CHUNK:(k + 1) * CHUNK])
                        nc.sync.dma_start(
                            xt[64:72, :],
                            base_d[hf].ap()[:, k * CHUNK:(k + 1) * CHUNK])

            for t in range(nsteps):
                for m in range(NM):
                    for k in range(NCHUNK):
                        cs = slice(0, CHUNK)
                        R = rpool.tile([128, CHUNK], f32, tag='r')
                        # open the R accumulation group with 64*dX
                        for hf in range(2):
                            nc.tensor.matmul(
                                R[:], seli[:, hf * 128:(hf + 1) * 128],
                                X[m][hf][k][0:64, cs],
                                start=(hf == 0), stop=False,
                                skip_group_check=True)

                        pend = []

                        def emit_m4(flush=False):
                            lag = 0 if flush else 2
                            while len(pend) > lag:
                                P, pr0, c = pend.pop(0)
                                for h in (0, 1):
                                    s = 2 * pr0 + h
                                    scol = (s * 2 + c) * 128
                                    rhs = P[:, h * 512:h * 512 + CHUNK]
                                    last = flush and not pend and h == 1
                                    nc.tensor.matmul(
                                        R[:], selq[:, scol:scol + 128], rhs,
                                        start=False, stop=last,
                                        skip_group_check=True)

                        for pr in range(4):
                            pr2 = pr // 2
                            win = X[m][pr2][k][0:72, cs]
                            WW = []
                            for uv in (1, 0):     # 0 = u (x, c=0 rows), 1 = v (y)
                                # 1024-wide so each half sits in its own psum bank
                                D = dap.tile([128, 1024], f32, tag='da')
                                for h in (0, 1):
                                    s = 2 * pr + h
                                    v = (s % 4) * 2 + uv
                                    nc.tensor.matmul(
                                        D[:, h * 512:h * 512 + CHUNK],
                                        l1b[0:72, v * 128:(v + 1) * 128],
                                        win, start=True, stop=True,
                                        skip_group_check=True)
                                AV = avp.tile([128, 2 * CHUNK], bf16, tag='av')
                                Dv = D[:].rearrange("p (h w) -> p h w", h=2)[:, :, 0:CHUNK]
                                nc.scalar.activation(AV[:], Dv, Abs, bias=bias[:], scale=1.0)
                                WW.append(AV)
                            WV, WU = WW
                            for c in (0, 1):
                                # both h into one A tile: same stationary tbl_c
                                A = dap.tile([128, 1024], f32, tag='da')
                                tcol = ((t * NM + m) * 2 + c) * 128
                                for h in (0, 1):
                                    nc.tensor.matmul(
                                        A[:, h * 512:h * 512 + CHUNK],
                                        tbl[:, tcol:tcol + 128],
                                        WV[:, h * CHUNK:(h + 1) * CHUNK],
                                        start=True, stop=True)
                                # merged product over both h: P = A * AVU
                                P = pp.tile([128, 1024], bf16, tag='p')
                                Av = A[:].rearrange("p (h2 w) -> p h2 w", h2=2)[:, :, 0:CHUNK]
                                Pv = P[:].rearrange("p (h2 w) -> p h2 w", h2=2)[:, :, 0:CHUNK]
                                wu = WU[:].rearrange("p (h2 w) -> p h2 w", h2=2)
                                nc.vector.tensor_tensor(Pv, Av, wu, Alu.mult)
                                pend.append((P, pr, c))
                                emit_m4()
                        emit_m4(flush=True)
                        for hf in range(2):
                            nc.scalar.activation(
                                X[m][hf][k][0:64, cs],
                                R[64 * hf:64 * hf + 64, cs],
                                Copy, bias=0.0, scale=1.0 / PSC)

            for m in range(NM):
                for hf in range(2):
                    for k in range(NCHUNK):
                        nc.sync.dma_start(
                            xout_d[m].ap()[64 * hf:64 * hf + 64,
                                           k * CHUNK:(k + 1) * CHUNK],
                            X[m][hf][k][0:64, :])

    nc.compile()
    return nc


def _get_compiled():
    global _COMPILED
    if _COMPILED is None:
        _COMPILED = _build_kernel()
    return _COMPILED


# ------------------------------------------------------------- host tensors
def _cmat():
    C = np.zeros((DG, DG), np.float32)
    for kk in range(DG):
        for jj in (kk - 1, kk, kk + 1):
            if 1 <= jj <= DG - 2:
                C[kk, jj] = 0.5 * (-2.0 if jj == kk else 1.0)
    return C


def _host_inputs(inputs):
    v_raw = _velocity_tables(inputs)   # [10, B, i, j, c]
    tp = inputs['template_points'].astype(np.float32)

    # abs-basis transform: hat_j(v) = (|v-(j-1)| - 2|v-j| + |v-(j+1)|)/2 for
    # j=1..14; hat_0/hat_15 vanish on the coord range, so their rows/cols drop.
    C = _cmat()
    v_all = np.einsum('ai,bj,tnijc->tnabc', C, C, v_raw).astype(np.float32)

    # ---- step t=0 on host: positions are the template for every batch ----
    u = 3.0 * tp[:, 0] + 7.5
    v = 3.0 * tp[:, 1] + 7.5
    i0 = np.clip(np.floor(u), 0, DG - 1).astype(np.int64)
    j0 = np.clip(np.floor(v), 0, DG - 1).astype(np.int64)
    i1 = np.clip(i0 + 1, 0, DG - 1)
    j1 = np.clip(j0 + 1, 0, DG - 1)
    fu = (u - i0)[None, :, None]
    fv = (v - j0)[None, :, None]
    vf = v_raw[0].reshape(B, DG * DG, 2)
    dx0 = DT * ((vf[:, i0 * DG + j0] * (1 - fu) * (1 - fv)
                 + vf[:, i0 * DG + j1] * (1 - fu) * fv
                 + vf[:, i1 * DG + j0] * fu * (1 - fv)
                 + vf[:, i1 * DG + j1] * fu * fv))   # [B, NPTS, 2]
    dx0 = _to_bf16(dx0).astype(np.float32)

    # base rows: base[hf][(s%4)*2 + c, j] = 3*x0[(4*hf + s%4)*W + j, c]
    u0 = 3.0 * tp                       # [NPTS, 2]
    bases = []
    for hf in range(2):
        bh = np.zeros((8, W), np.float32)
        for s4 in range(4):
            for c in range(2):
                p0 = (4 * hf + s4) * W
                bh[s4 * 2 + c] = u0[p0:p0 + W, c]
        bases.append(_to_bf16(bh))

    # mm_a stationary variants, K=72 (rows 0..63: s%4, c, g; rows 64..71 base):
    # L1B[(v//2)*16 + (v%2)*8 + g, v*128 + g*16 + j] = 3, and
    # L1B[64 + v, v*128 + :] = 1  (injects base row (s%4, c=uv))
    # where variant v = (s%4)*2 + uv  (uv: 0 = u rows (c=0), 1 = v rows (c=1))
    l1b = np.zeros((72, 8 * 128), np.float32)
    for vv in range(8):
        roff = (vv // 2) * 16 + (vv % 2) * 8
        for g in range(8):
            l1b[roff + g, vv * 128 + g * 16:vv * 128 + g * 16 + 16] = 3.0
        l1b[64 + vv, vv * 128:(vv + 1) * 128] = 1.0

    biasv = np.zeros((128, 1), np.float32)
    biasv[:, 0] = 7.5 - (np.arange(128) % 16)

    # m4 stationaries: SELQ[(g*16+i), (s*2+c)*128 + (s*16+c*8+g)] = 1
    selq = np.zeros((128, 16 * 128), np.float32)
    for s in range(8):
        for c in range(2):
            base = (s * 2 + c) * 128
            for g in range(8):
                selq[g * 16:(g + 1) * 16, base + s * 16 + c * 8 + g] = 1.0

    # identity-add stationaries: R[64*hf + r] += PSC * xt[r]
    seli = np.zeros((64, 2 * 128), np.float32)
    for hf in range(2):
        for r in range(64):
            seli[r, hf * 128 + 64 * hf + r] = PSC

    # per-core block-diag tables for device steps t=1..9 (scaled by PSC)
    # TBL[(g*16+j), ((t*NM+m)*2+c)*128 + g*16+i] = PSC * DT * velC[b][i, j, c]
    vv_ = v_all[1:].reshape(NSTEPS_DEV, NCORES, NM, G, DG, DG, 2)
    tbls = []
    for core in range(NCORES):
        tblc = np.zeros((NSTEPS_DEV, NM, 2, G, 16, G, 16), np.float32)
        for g in range(G):
            tblc[:, :, :, g, :, g, :] = (vv_[:, core, :, g].transpose(0, 1, 4, 3, 2)
                                         * (DT * PSC))
        tbl = tblc.transpose(3, 4, 0, 1, 2, 5, 6).reshape(128, NSTEPS_DEV * NM * 2 * 128)
        tbls.append(_to_bf16(tbl))

    # per-core dx0 in X-tile layout: dx0_m[s*16+c*8+g, s-col] for batch core*32+m*8+g
    dx0s = []
    for core in range(NCORES):
        percore = []
        for m in range(NM):
            dm = np.zeros((128, W), np.float32)
            for g in range(G):
                b = core * BC + m * G + g
                r = dx0[b].reshape(8, W, 2)          # [s, w, c]
                for s in range(8):
                    for c in range(2):
                        dm[s * 16 + c * 8 + g] = r[s, :, c]
            percore.append(_to_bf16(dm))
        dx0s.append(percore)

    return tbls, bases, _to_bf16(l1b), biasv, _to_bf16(selq), _to_bf16(seli), dx0s, tp


LAST_RES = None


def kernel(**inputs):
    global LAST_RES
    import os
    inputs = {k: np.asarray(v) for k, v in inputs.items()}
    from concourse.bass_utils import run_bass_kernel_spmd

    nc = _get_compiled()
    tbls, bases, l1b, biasv, selq, seli, dx0s, tp = _host_inputs(inputs)

    in_maps = []
    for core in range(NCORES):
        im = {'tbl': tbls[core], 'base0': bases[0], 'base1': bases[1],
              'l1b': l1b, 'bias': biasv, 'selq': selq, 'seli': seli}
        for m in range(NM):
            im[f'dx0_{m}'] = dx0s[core][m]
        in_maps.append(im)
    tmpdir = os.environ.get('BASS_TRACE_DIR') or None
    if tmpdir:
        os.makedirs(tmpdir, exist_ok=True)
    res = run_bass_kernel_spmd(nc, in_maps, list(range(NCORES)), tmpdir=tmpdir)
    LAST_RES = res

    out = np.empty((B, NPTS, 2), np.float32)
    for core in range(NCORES):
        for m in range(NM):
            xm = np.asarray(res.results[core][f'xout{m}']).astype(np.float32)
            rm = xm.reshape(8, 2, 8, W)                         # [s, c, g, w]
            b0 = core * BC + m * G
            out[b0:b0 + G] = tp[None] + rm.transpose(2, 0, 3, 1).reshape(G, NPTS, 2)
    return out
